# revision 1
# baseline (speedup 1.0000x reference)
"""LlamaAttention forward on 8 Trainium2 NeuronCores (Bass/Tile).

Sharding: core = b * 4 + g  (b = batch 0/1, g = head-group 0..3, 4 heads each).
Each core computes q/k/v projections + RoPE + causal attention for its 4 heads
of its batch, then a partial output projection against its slice of wo.
The host sums the 4 partial outputs per batch (exact, fp64 accumulate).

All matmuls run in float32r (TF32-like PE mode: full speed, ~2e-4 rel err).

Layout notes (per core, S=2048, H=2048, M=512 head-width):
  xT   [H, S]   hidden.T             -> rhs of q/k projections, lhsT of v proj
  wqT  [H, M]   (wq_slice/sqrt(hd)).T-> lhsT of q proj  (scores scale folded in)
  qT/kT [M, S]  (d on partitions)    -> RoPE along free dim + partition-swap,
                                        then lhsT/rhs of scoresT = k @ q.T
  scoresT [sk, sq] (psum)            -> +mask, exp (ACT) -> expT [sk, sq] f32r
  pvT  [d, sq] = v.T @ expT          -> v [sk, d] is lhsT, expT moving
  sums over sk via ones-matmul; reciprocal broadcast back via K=1 matmul
  attnT [d(=m), sq]                  -> lhsT of out = attn @ wo_slice.T
"""

import os
import numpy as np
from contextlib import ExitStack

import concourse.bacc as bacc
import concourse.tile as tile
import concourse.mybir as mybir
from concourse.bass_utils import run_bass_kernel_spmd

F32 = mybir.dt.float32
F32R = mybir.dt.float32r
EXP = mybir.ActivationFunctionType.Exp
MULT = mybir.AluOpType.mult

NCORES = 8
B = 2
HD = 128
NEG = -1.0e30

# filled in by kernel() / test harness
LAST_RESULTS = None


def _build(S, H, M, causal, n_cores=NCORES, phases=(1, 2, 3)):
    """Build + compile the per-core program. M = heads_per_core * 128."""
    P = 128
    NKO = H // P          # h-tiles (contraction) for projections
    NMT = M // P          # heads per core
    QC = 256              # qkv s-chunk (moving free dim)
    NQC = S // QC
    SC = 512              # attention sq-chunk
    NSC = S // SC
    NST = S // P          # sk tiles / sq tiles
    assert NQC >= 2

    nc = bacc.Bacc("TRN2", target_bir_lowering=False, debug=False,
                   num_devices=n_cores)

    xT = nc.dram_tensor("xT", [H, S], F32R, kind="ExternalInput").ap()
    wqT = nc.dram_tensor("wqT", [H, M], F32R, kind="ExternalInput").ap()
    wkT = nc.dram_tensor("wkT", [H, M], F32R, kind="ExternalInput").ap()
    wvT = nc.dram_tensor("wvT", [H, M], F32R, kind="ExternalInput").ap()
    woT = nc.dram_tensor("woT", [M, H], F32R, kind="ExternalInput").ap()
    trig = nc.dram_tensor("trig", [P, 2, S], F32, kind="ExternalInput").ap()
    if causal:
        diag_d = nc.dram_tensor("diag", [P, SC // P, SC], F32,
                                kind="ExternalInput").ap()
    else:
        maskT_d = nc.dram_tensor("maskT", [S, S], F32, kind="ExternalInput").ap()
    outp = nc.dram_tensor("outp", [S, H], F32, kind="ExternalOutput").ap()

    xT_r = xT.rearrange("(ko p) s -> p ko s", p=P)
    wq_r = wqT.rearrange("(ko p) m -> p ko m", p=P)
    wk_r = wkT.rearrange("(ko p) m -> p ko m", p=P)
    wv_r = wvT.rearrange("(ko p) m -> p ko m", p=P)
    wo_r = woT.rearrange("(mt p) o -> p mt o", p=P)

    with tile.TileContext(nc) as tc, ExitStack() as top, \
         nc.allow_low_precision(reason="float32r is 4-byte; rounding for PE fp32r mode"):
        persist = top.enter_context(tc.tile_pool(name="persist", bufs=1))
        dramp = top.enter_context(tc.tile_pool(name="dram", bufs=1, space="DRAM"))
        ps_mm = top.enter_context(tc.tile_pool(name="ps_mm", bufs=4, space="PSUM"))
        ps_pv = top.enter_context(tc.tile_pool(name="ps_pv", bufs=2, space="PSUM"))
        ps_sm = top.enter_context(tc.tile_pool(name="ps_sm", bufs=1, space="PSUM"))
        ps_bc = top.enter_context(tc.tile_pool(name="ps_bc", bufs=1, space="PSUM"))

        # DRAM scratch for rope'd qT/kT between phases (one tile per
        # SC-chunk so reload deps are fine-grained); v stays in SBUF
        qT_ds, kT_ds = [], []
        for i in range(NSC):
            qt = dramp.tile([M, SC], F32R, name="qT_d")
            qT_ds.append(qt[:].rearrange("(mt p) s -> p mt s", p=P))
            kt = dramp.tile([M, SC], F32R, name="kT_d")
            kT_ds.append(kt[:].rearrange("(mt p) s -> p mt s", p=P))
        v_sb = persist.tile([P, NST, M], F32R)   # v: [sk%128, sk//128, m]

        # reserved early tiles: q/k chunk-0 for heads 0-1, loadable while
        # phase-1 SBUF is still live (their addresses never overlap phase 1)
        NE = min(3, NMT) if causal else 0  # heads w/ reserved early tiles
        if 2 in phases and causal:
            q0e = persist.tile([P, NE, SC], F32R, name="q0e")
            k0e = persist.tile([P, NE, SC], F32R, name="k0e")
            diag = persist.tile([P, SC // P, SC], F32)

        ones_f = persist.tile([P, 1], F32)
        nc.vector.memset(ones_f[:], 1.0)
        ones_col = persist.tile([P, 1], F32R)          # lhsT [K=128, M=1]
        nc.vector.tensor_copy(ones_col[:], ones_f[:])
        ones_row = persist.tile([1, P], F32R)          # lhsT [K=1, M=128]
        nc.vector.tensor_copy(ones_row[:], ones_f[0:1, 0:1].to_broadcast([1, P]))

        # ---------------- Phase 1: QKV projections + RoPE ----------------
        if 1 in phases:
         with tc.tile_pool(name="wq", bufs=1) as wqp, \
             tc.tile_pool(name="wk", bufs=1) as wkp, \
             tc.tile_pool(name="wv", bufs=1) as wvp, \
             tc.tile_pool(name="xc", bufs=2) as xcp, \
             tc.tile_pool(name="trig", bufs=2) as trigp, \
             tc.tile_pool(name="ropetmp", bufs=2) as rtp, \
             tc.tile_pool(name="stage", bufs=2) as stp:

            wq_sb = wqp.tile([P, NKO, M], F32R)
            wk_sb = wkp.tile([P, NKO, M], F32R)
            wv_sb = wvp.tile([P, NKO, M], F32R)
            xcs = {}
            # DMA order: x chunk 0, then wq per head-slice (so the first
            # q chains start after ~3MB), then wk; wv+x1 stream under compute.
            trigs = {}

            def load_trig(c):
                trigs[c] = trigp.tile([P, 2, QC], F32, tag="trig", name="tg")
                nc.sync.dma_start(trigs[c][:], trig[:, :, c * QC:(c + 1) * QC])

            xcs[0] = xcp.tile([P, NKO, QC], F32R, tag="xc", name="xc0")
            nc.sync.dma_start(xcs[0][:], xT_r[:, :, 0:QC])
            for mt in range(NMT):
                ms = slice(mt * P, (mt + 1) * P)
                nc.sync.dma_start(wq_sb[:, :, ms], wq_r[:, :, ms])
            load_trig(0)
            for mt in range(NMT):
                ms = slice(mt * P, (mt + 1) * P)
                nc.sync.dma_start(wk_sb[:, :, ms], wk_r[:, :, ms])

            def load_xc(c):
                xcs[c] = xcp.tile([P, NKO, QC], F32R, tag="xc", name="xc")
                nc.sync.dma_start(xcs[c][:], xT_r[:, :, c * QC:(c + 1) * QC])
                if c not in trigs:
                    load_trig(c)

            def qk_proj(c):
                tg = trigs.pop(c)
                cs2 = slice((c * QC) % SC, (c * QC) % SC + QC)
                for w_sb, dsts in ((wq_sb, qT_ds), (wk_sb, kT_ds)):
                    dst = dsts[c * QC // SC]
                    staged = stp.tile([P, NMT, QC], F32R, tag="st", name="staged")
                    for mt in range(NMT):
                        ps_full = ps_mm.tile([P, SC], F32, tag="mm", name="ps_qk")
                        ps = ps_full[:, :QC]
                        for ko in range(NKO):
                            nc.tensor.matmul(
                                ps, w_sb[:, ko, mt * P:(mt + 1) * P],
                                xcs[c][:, ko, :],
                                start=(ko == 0), stop=(ko == NKO - 1))
                        # RoPE: out = ps*cos + swap64(ps)*sin_signed
                        t1 = rtp.tile([P, QC], F32, tag="t1")
                        nc.vector.tensor_tensor(t1[:], ps, tg[:, 0, :], MULT)
                        t2 = rtp.tile([P, QC], F32, tag="t2")
                        nc.vector.tensor_tensor(t2[0:64, :], ps[64:128, :],
                                                tg[0:64, 1, :], MULT)
                        nc.vector.tensor_tensor(t2[64:128, :], ps[0:64, :],
                                                tg[64:128, 1, :], MULT)
                        nc.vector.tensor_add(staged[:, mt, :], t1[:], t2[:])
                    nc.sync.dma_start(dst[:, :, cs2], staged[:])

            def v_proj(c):
                for st in range(QC // P):
                    ps_full = ps_mm.tile([P, SC], F32, tag="mm", name="ps_v")
                    ps = ps_full[:, :M]
                    for ko in range(NKO):
                        nc.tensor.matmul(
                            ps, xcs[c][:, ko, st * P:(st + 1) * P],
                            wv_sb[:, ko, :],
                            start=(ko == 0), stop=(ko == NKO - 1))
                    nc.vector.tensor_copy(v_sb[:, c * (QC // P) + st, :], ps)

            # c=0/1: q,k only (weights for v still in flight), then v(0), v(1)
            qk_proj(0)
            load_xc(1)
            nc.sync.dma_start(wv_sb[:], wv_r)
            qk_proj(1)
            if 2 in phases and causal:
                # prefetch q/k chunk-0 lead heads into reserved tiles; deps
                # (chunk 0/1 spills) are already satisfied here
                nc.scalar.dma_start(diag[:], diag_d[:])
                for h in range(NE):
                    nc.scalar.dma_start(q0e[:, h], qT_ds[0][:, h])
                    nc.gpsimd.dma_start(k0e[:, h], kT_ds[0][:, h])
            v_proj(0)
            v_proj(1)
            del xcs[0]
            for c in range(2, NQC):
                load_xc(c)
                qk_proj(c)
                v_proj(c)
                del xcs[c - 1]

        # -------- Phase 2+3: attention (sq-chunk outer) fused with o-proj ----
        if 2 in phases:
         with tc.tile_pool(name="wo", bufs=1) as wop, \
             tc.tile_pool(name="kh", bufs=1) as khp, \
             tc.tile_pool(name="qh", bufs=2) as qhp, \
             tc.tile_pool(name="exp", bufs=4) as epp, \
             tc.tile_pool(name="smx", bufs=3) as smp, \
             tc.tile_pool(name="attc", bufs=3) as attp, \
             tc.tile_pool(name="oout", bufs=3) as oop, \
             (tc.tile_pool(name="mskt", bufs=3) if not causal else ExitStack()) as mtp:

            if causal and 1 not in phases:
                nc.gpsimd.dma_start(diag[:], diag_d[:])

            # DMA order: q for chunk 0 (all heads), k sk-slices (early tiles
            # first), wo after the k tiles chunks 0/1 need, rest of k.
            q_tiles = {}

            def load_q(c, engine):
                t_ = qhp.tile([P, NMT, SC], F32R, tag="qh", name="q_t")
                engine.dma_start(t_[:], qT_ds[c])
                q_tiles[c] = t_

            # fine-grained preamble: per-head q/k slices for chunk 0 in the
            # order the first scores matmuls consume them, then the rest.
            k_sb = khp.tile([P, NMT, S], F32R)   # kT: [d, head, sk]
            q0 = qhp.tile([P, NMT, SC], F32R, tag="qh", name="q_t0")
            q_tiles[0] = q0
            for h in range(NE, NMT):
                nc.scalar.dma_start(q0[:, h], qT_ds[0][:, h])
                nc.gpsimd.dma_start(k_sb[:, h, 0:SC], kT_ds[0][:, h])
            if NE > 0:
                nc.gpsimd.dma_start(k_sb[:, 0:NE, 0:SC], kT_ds[0][:, 0:NE])
            n_kc = (2 if causal else NSC) if NSC >= 2 else 1
            for kc in range(1, n_kc):
                nc.gpsimd.dma_start(
                    k_sb[:, :, kc * SC:(kc + 1) * SC], kT_ds[kc])
            wo_sb = wop.tile([P, NMT, H], F32R)
            for mt in range(NMT):
                nc.sync.dma_start(wo_sb[:, mt], wo_r[:, mt])
            for kc in range(n_kc, NSC):
                nc.gpsimd.dma_start(
                    k_sb[:, :, kc * SC:(kc + 1) * SC], kT_ds[kc])

            for c in range(NSC):
                cq = slice(c * SC, (c + 1) * SC)
                nt = min(NST, (c + 1) * SC // P) if causal else NST
                if c + 1 < NSC:
                    load_q(c + 1, nc.sync)
                q_all = q_tiles.pop(c)
                attn_c = attp.tile([P, NMT, SC], F32R, tag="attc", name="attn_c")
                for h in range(NMT):
                    hs = slice(h * P, (h + 1) * P)
                    early = causal and c == 0 and h < NE
                    q_src = q0e[:, h, :] if early else q_all[:, h, :]
                    pv = ps_pv.tile([P, SC], F32, tag="pv")
                    psum_s = ps_sm.tile([1, SC], F32, tag="sums")
                    for t in range(nt):
                        if early:
                            k_src = k0e[:, h, t * P:(t + 1) * P]
                        else:
                            k_src = k_sb[:, h, t * P:(t + 1) * P]
                        ss = ps_mm.tile([P, SC], F32, tag="mm")
                        nc.tensor.matmul(ss, k_src,
                                         q_src, start=True, stop=True)
                        if causal:
                            j = t - (c * SC // P)
                            if j >= 0:  # diagonal tile: causal pattern on the
                                w = min(SC, (j + 1) * P)  # columns it touches
                                nc.vector.tensor_add(ss[:, :w], ss[:, :w],
                                                     diag[:, j, :w])
                        else:
                            mt_t = mtp.tile([P, SC], F32, tag="mask")
                            nc.sync.dma_start(
                                mt_t[:], maskT_d[t * P:(t + 1) * P, cq])
                            nc.vector.tensor_add(ss, ss, mt_t[:])
                            nc.vector.tensor_scalar_max(ss, ss, -3.0e38)
                        e = epp.tile([P, SC], F32R, tag="e")
                        nc.scalar.activation(e[:], ss, EXP)
                        nc.tensor.matmul(pv, v_sb[:, t, hs], e[:],
                                         start=(t == 0), stop=(t == nt - 1),
                                         skip_group_check=True)
                        nc.tensor.matmul(psum_s, ones_col[:], e[:],
                                         start=(t == 0), stop=(t == nt - 1),
                                         skip_group_check=True)
                    rec = smp.tile([1, SC], F32R, tag="rec")
                    nc.vector.reciprocal(rec[:], psum_s)
                    bc = ps_bc.tile([P, SC], F32, tag="bc")
                    nc.tensor.matmul(bc, ones_row[:], rec[:],
                                     start=True, stop=True)
                    rb = smp.tile([P, SC], F32, tag="rb")
                    nc.vector.tensor_copy(rb[:], bc)
                    nc.vector.tensor_tensor(attn_c[:, h, :], pv, rb[:], MULT)

                # o-proj for this sq chunk (all heads of attn_c ready)
                if 3 in phases:
                    for st in range(SC // P):
                        rs = slice(c * SC + st * P, c * SC + (st + 1) * P)
                        o_st = oop.tile([P, H], F32, tag="o_st", name="o_st")
                        for oc in range(H // SC):
                            ps = ps_mm.tile([P, SC], F32, tag="mm", name="ps_o")
                            for mt in range(NMT):
                                nc.tensor.matmul(
                                    ps, attn_c[:, mt, st * P:(st + 1) * P],
                                    wo_sb[:, mt, oc * SC:(oc + 1) * SC],
                                    start=(mt == 0), stop=(mt == NMT - 1))
                            nc.vector.tensor_copy(
                                o_st[:, oc * SC:(oc + 1) * SC], ps)
                            if oc % 2 == 1:  # flush staged half early
                                hs_o = slice((oc - 1) * SC, (oc + 1) * SC)
                                nc.sync.dma_start(outp[rs, hs_o],
                                                  o_st[:, hs_o])

    nc.compile()
    return nc


_CACHE = {}


def _get_program(S, H, M, causal, phases=(1, 2, 3)):
    key = (S, H, M, causal, phases)
    if key not in _CACHE:
        _CACHE[key] = _build(S, H, M, causal, phases=phases)
    return _CACHE[key]


def _ensure_ntff_hook():
    """Register the axon NTFF profile hook if the environment lacks
    antenv.axon_hooks (trace support). Returns True if tracing is usable."""
    import sys
    import types
    try:
        from antenv.axon_hooks import get_axon_ntff_profile_hook
        return get_axon_ntff_profile_hook() is not None
    except ImportError:
        pass
    try:
        from trn_agent_boot.trn_boot import _ntff_profile_via_ctypes
        hook = _ntff_profile_via_ctypes("/opt/axon/libaxon_pjrt.so")
        if hook is None:
            return False
        m = types.ModuleType("antenv.axon_hooks")
        m._hook = hook
        m.set_axon_ntff_profile_hook = lambda h: setattr(m, "_hook", h)
        m.get_axon_ntff_profile_hook = lambda: m._hook
        sys.modules["antenv.axon_hooks"] = m
        import antenv
        antenv.axon_hooks = m
        return True
    except Exception:
        return False


def _rope_tables(S, dim, base=10000.0):
    inv_freq = 1.0 / (base ** (np.arange(0, dim, 2, dtype=np.float64) / dim))
    t = np.arange(S, dtype=np.float64)
    freqs = np.outer(t, inv_freq)                     # [S, dim/2]
    emb = np.concatenate([freqs, freqs], axis=-1)     # [S, dim]
    return (np.cos(emb).astype(np.float32), np.sin(emb).astype(np.float32))


def kernel(hidden_states, attention_mask, position_ids, wq, wk, wv, wo):
    global LAST_RESULTS
    hidden_states = np.asarray(hidden_states, dtype=np.float32)
    attention_mask = np.asarray(attention_mask, dtype=np.float32)
    position_ids = np.asarray(position_ids)
    wq = np.asarray(wq, dtype=np.float32)
    wk = np.asarray(wk, dtype=np.float32)
    wv = np.asarray(wv, dtype=np.float32)
    wo = np.asarray(wo, dtype=np.float32)

    Bq, S, H = hidden_states.shape
    assert Bq == B and H % HD == 0
    nh = H // HD
    groups = NCORES // B                     # head-groups per batch
    hpg = nh // groups                       # heads per core
    M = hpg * HD

    # causal-mask detection (exact match against the standard Llama pattern)
    neg = np.finfo(np.float32).min
    causal_ref = np.where(np.tril(np.ones((S, S), dtype=bool)), np.float32(0.0),
                          np.float32(neg))
    causal = all(np.array_equal(attention_mask[b, 0], causal_ref)
                 for b in range(B))

    cos_tab, sin_tab = _rope_tables(S, HD)
    scale = 1.0 / np.sqrt(HD)

    SC = 512
    P = 128
    in_maps = []
    for core in range(NCORES):
        b, g = divmod(core, groups)
        rows = slice(g * M, (g + 1) * M)
        x = hidden_states[b]                                   # [S, H]
        pos = position_ids[b].astype(np.int64)
        cosT = cos_tab[pos].T                                  # [HD, S]
        sinT = sin_tab[pos].T
        sinS = np.concatenate([-sinT[:HD // 2], sinT[HD // 2:]], axis=0)
        m = {
            "xT": np.ascontiguousarray(x.T),
            "wqT": np.ascontiguousarray((wq[rows].astype(np.float64) * scale)
                                        .astype(np.float32).T),
            "wkT": np.ascontiguousarray(wk[rows].T),
            "wvT": np.ascontiguousarray(wv[rows].T),
            "woT": np.ascontiguousarray(wo[:, rows].T),
            "trig": np.ascontiguousarray(
                np.stack([cosT, sinS], axis=1).astype(np.float32)),
        }
        if causal:
            p_idx = np.arange(P)[:, None, None]
            j_idx = np.arange(SC // P)[None, :, None]
            f_idx = np.arange(SC)[None, None, :]
            m["diag"] = np.where(p_idx + P * j_idx <= f_idx,
                                 np.float32(0.0), np.float32(NEG)).astype(np.float32)
        else:
            m["maskT"] = np.ascontiguousarray(attention_mask[b, 0].T)
        in_maps.append(m)

    nc = _get_program(S, H, M, causal)
    globals()["LAST_IN_MAPS"] = in_maps
    trace = os.environ.get("TRN_ATTN_TRACE", "") == "1"
    if trace:
        trace = _ensure_ntff_hook()
    res = run_bass_kernel_spmd(nc, in_maps, core_ids=list(range(NCORES)),
                               trace=trace)
    LAST_RESULTS = res

    out = np.zeros((B, S, H), dtype=np.float64)
    for core in range(NCORES):
        b = core // groups
        out[b] += res.results[core]["outp"].astype(np.float64)
    return out.astype(np.float32)



# revision 2
# speedup vs baseline: 6.4679x; 6.4679x over previous
"""LlamaAttention forward on 8 Trainium2 NeuronCores (Bass/Tile), v2.

Sharding: core = b * 4 + g  (b = batch 0/1, g = head-group 0..3, 4 heads each).
Each core computes q/k/v projections + RoPE + causal attention for its 4 heads
of its batch, then a partial output projection against its slice of wo.
The host sums the 4 partial outputs per batch (exact, fp64 accumulate).

v2 changes vs v1:
  - All matmul operands bf16 (fp32 PSUM accumulate): halves HBM traffic and
    SBUF footprint; measured metric vs fp32 reference ~3.4e-3 (limit 2e-2).
  - q/k/v stay SBUF-resident between projection and attention (no DRAM
    spill/reload round-trip).
  - Projection s-chunks are 512 wide (LDWEIGHTS fully hidden under N=512
    moving streams; N=256 exposed it).
  - Softmax denominator via an all-ones [128,128] stationary matmul into a
    [128,512] PSUM (every partition holds the column sums), removing the
    reciprocal-broadcast matmul and its copy.
  - Causal diagonal blocks compute only the valid column range (off = j*128)
    in scores / exp / pv / sums.
  - scores(t) issue runs 2 tiles ahead of pv(t)/sums(t) so the ACT exp
    latency never stalls the PE.

Layout notes (per core, S=2048, H=2048, M=512 head-width):
  xT   [H, S]  bf16 hidden.T       -> rhs of q/k projections, lhsT of v proj
  wqT  [H, M]  bf16 (wq/sqrt(hd)).T-> lhsT of q proj (scores scale folded in)
  qT/kT [d, h, S] bf16 SBUF        -> RoPE'd; lhsT/rhs of scoresT = k @ q.T
  scoresT [sk, sq] (psum)          -> +diag mask, exp (ACT) -> e bf16
  pv   [d, sq] psum += v.T @ e     -> v [sk, d] is lhsT, e moving
  sums [*, sq] psum += ones @ e    -> all partitions hold column sums
  attn = pv * 1/sums (DVE)         -> bf16, lhsT of out = attn @ wo_slice.T
"""

import os
import numpy as np
from contextlib import ExitStack

import ml_dtypes

import concourse.bacc as bacc
import concourse.tile as tile
import concourse.mybir as mybir
from concourse.bass_utils import run_bass_kernel_spmd

F32 = mybir.dt.float32
BF16 = mybir.dt.bfloat16
EXP = mybir.ActivationFunctionType.Exp
COPY = mybir.ActivationFunctionType.Copy
MULT = mybir.AluOpType.mult

NCORES = 8
B = 2
HD = 128
NEG = -1.0e30

# filled in by kernel() / test harness
LAST_RESULTS = None


def _build(S, H, M, causal, n_cores=NCORES, repeat=1):
    """Build + compile the per-core program. M = heads_per_core * 128."""
    P = 128
    NKO = H // P          # h-tiles (contraction) for projections
    NMT = M // P          # heads per core
    QC = 512              # qkv s-chunk (moving free dim)
    NQC = S // QC
    SC = 512              # attention sq-chunk
    NSC = S // SC
    NST = S // P          # sk tiles
    DJ = SC // P          # diagonal sub-tiles per chunk

    nc = bacc.Bacc("TRN2", target_bir_lowering=False, debug=False,
                   num_devices=n_cores)

    xT = nc.dram_tensor("xT", [H, S], BF16, kind="ExternalInput").ap()
    wqT = nc.dram_tensor("wqT", [H, M], BF16, kind="ExternalInput").ap()
    wkT = nc.dram_tensor("wkT", [H, M], BF16, kind="ExternalInput").ap()
    wvT = nc.dram_tensor("wvT", [H, M], BF16, kind="ExternalInput").ap()
    woT = nc.dram_tensor("woT", [M, H], BF16, kind="ExternalInput").ap()
    trig = nc.dram_tensor("trig", [P, 2, S], F32, kind="ExternalInput").ap()
    if causal:
        diag_d = nc.dram_tensor("diag", [P, DJ, SC], F32,
                                kind="ExternalInput").ap()
    else:
        maskT_d = nc.dram_tensor("maskT", [S, S], F32, kind="ExternalInput").ap()
    outp = nc.dram_tensor("outp", [S, H], F32, kind="ExternalOutput").ap()

    xT_r = xT.rearrange("(ko p) s -> p ko s", p=P)
    wq_r = wqT.rearrange("(ko p) m -> p ko m", p=P)
    wk_r = wkT.rearrange("(ko p) m -> p ko m", p=P)
    wv_r = wvT.rearrange("(ko p) m -> p ko m", p=P)
    wo_r = woT.rearrange("(mt p) o -> p mt o", p=P)

    with tile.TileContext(nc) as tc, ExitStack() as top, \
         nc.allow_low_precision(reason="bf16 operands; fp32 PSUM accumulate"):
        persist = top.enter_context(tc.tile_pool(name="persist", bufs=1))
        ps_mm = top.enter_context(tc.tile_pool(name="ps_mm", bufs=4, space="PSUM"))
        ps_pv = top.enter_context(tc.tile_pool(name="ps_pv", bufs=2, space="PSUM"))
        ps_sm = top.enter_context(tc.tile_pool(name="ps_sm", bufs=2, space="PSUM"))

        qT_sb = persist.tile([P, NMT, S], BF16)   # [d, head, s]
        kT_sb = persist.tile([P, NMT, S], BF16)
        v_sb = persist.tile([P, NST, M], BF16)    # [sk%128, sk//128, m]
        ones_f = persist.tile([P, P], F32)
        ones_sb = persist.tile([P, P], BF16)      # lhsT for column sums
        if causal:
            diag_sb = persist.tile([P, DJ, SC], F32)

        for _rep in range(repeat):
            nc.vector.memset(ones_f[:], 1.0)
            nc.vector.tensor_copy(ones_sb[:], ones_f[:])

            # ---------------- Phase 1: QKV projections + RoPE ----------------
            with tc.tile_pool(name="wq", bufs=1) as wqp, \
                 tc.tile_pool(name="wk", bufs=1) as wkp, \
                 tc.tile_pool(name="wv", bufs=1) as wvp, \
                 tc.tile_pool(name="xc", bufs=2) as xcp, \
                 tc.tile_pool(name="trig", bufs=2) as trigp, \
                 tc.tile_pool(name="ropetmp", bufs=2) as rtp:

                wq_sb = wqp.tile([P, NKO, M], BF16)
                wk_sb = wkp.tile([P, NKO, M], BF16)
                wv_sb = wvp.tile([P, NKO, M], BF16)
                xcs = {}
                trigs = {}

                def load_trig(c):
                    trigs[c] = trigp.tile([P, 2, QC], F32, tag="trig", name="tg")
                    nc.sync.dma_start(trigs[c][:], trig[:, :, c * QC:(c + 1) * QC])

                def load_xc(c, engine=None):
                    xcs[c] = xcp.tile([P, NKO, QC], BF16, tag="xc", name="xc")
                    (engine or nc.sync).dma_start(
                        xcs[c][:], xT_r[:, :, c * QC:(c + 1) * QC])
                    if c not in trigs:
                        load_trig(c)

                # DMA order: x chunk 0, wq per head-slice (first q chain can
                # start early), trig0, wk, then wv + x1 stream under compute.
                load_xc(0)
                for mt in range(NMT):
                    ms = slice(mt * P, (mt + 1) * P)
                    nc.sync.dma_start(wq_sb[:, :, ms], wq_r[:, :, ms])
                load_trig(0)
                for mt in range(NMT):
                    ms = slice(mt * P, (mt + 1) * P)
                    nc.sync.dma_start(wk_sb[:, :, ms], wk_r[:, :, ms])
                if causal:
                    nc.scalar.dma_start(diag_sb[:], diag_d[:])

                def qk_proj(c):
                    tg = trigs.pop(c)
                    cs = slice(c * QC, (c + 1) * QC)
                    for w_sb, dst in ((wq_sb, qT_sb), (wk_sb, kT_sb)):
                        for mt in range(NMT):
                            ps = ps_mm.tile([P, QC], F32, tag="mm", name="ps_qk")
                            for ko in range(NKO):
                                nc.tensor.matmul(
                                    ps, w_sb[:, ko, mt * P:(mt + 1) * P],
                                    xcs[c][:, ko, :],
                                    start=(ko == 0), stop=(ko == NKO - 1))
                            # RoPE: out = ps*cos + swap64(ps)*sin_signed
                            t1 = rtp.tile([P, QC], F32, tag="t1")
                            nc.vector.tensor_tensor(t1[:], ps, tg[:, 0, :], MULT)
                            t2 = rtp.tile([P, QC], F32, tag="t2")
                            nc.vector.tensor_tensor(t2[0:64, :], ps[64:128, :],
                                                    tg[0:64, 1, :], MULT)
                            nc.vector.tensor_tensor(t2[64:128, :], ps[0:64, :],
                                                    tg[64:128, 1, :], MULT)
                            nc.vector.tensor_add(dst[:, mt, cs], t1[:], t2[:])

                def v_proj(c):
                    for st in range(QC // P):
                        ps = ps_mm.tile([P, M], F32, tag="mm", name="ps_v")
                        for ko in range(NKO):
                            nc.tensor.matmul(
                                ps, xcs[c][:, ko, st * P:(st + 1) * P],
                                wv_sb[:, ko, :],
                                start=(ko == 0), stop=(ko == NKO - 1))
                        nc.scalar.activation(
                            v_sb[:, c * (QC // P) + st, :], ps, COPY)

                # c=0: q,k only (wv still in flight), then v(0)
                qk_proj(0)
                load_xc(1)
                nc.sync.dma_start(wv_sb[:], wv_r)
                qk_proj(1)
                v_proj(0)
                v_proj(1)
                del xcs[0]
                for c in range(2, NQC):
                    load_xc(c)
                    qk_proj(c)
                    v_proj(c)
                    del xcs[c - 1]

            # -------- Phase 2+3: attention (sq-chunk outer) + o-proj --------
            with tc.tile_pool(name="wo", bufs=1) as wop, \
                 tc.tile_pool(name="exp", bufs=4) as epp, \
                 tc.tile_pool(name="smx", bufs=2) as smp, \
                 tc.tile_pool(name="attc", bufs=2) as attp, \
                 tc.tile_pool(name="oout", bufs=2) as oop, \
                 (tc.tile_pool(name="mskt", bufs=3) if not causal
                  else ExitStack()) as mtp:

                wo_sb = wop.tile([P, NMT, H], BF16)
                for mt in range(NMT):
                    nc.sync.dma_start(wo_sb[:, mt], wo_r[:, mt])

                for c in range(NSC):
                    cq = slice(c * SC, (c + 1) * SC)
                    nt = min(NST, (c + 1) * SC // P) if causal else NST
                    attn_c = attp.tile([P, NMT, SC], BF16, tag="attc",
                                       name="attn_c")
                    for h in range(NMT):
                        hs = slice(h * P, (h + 1) * P)
                        pv = ps_pv.tile([P, SC], F32, tag="pv")
                        sm = ps_sm.tile([P, SC], F32, tag="sums")

                        def issue_scores(t):
                            j = t - c * DJ
                            off = j * P if (causal and j >= 0) else 0
                            ss_full = ps_mm.tile([P, SC], F32, tag="mm")
                            ss = ss_full[:, off:]
                            nc.tensor.matmul(
                                ss, kT_sb[:, h, t * P:(t + 1) * P],
                                qT_sb[:, h, c * SC + off:(c + 1) * SC],
                                start=True, stop=True)
                            if causal:
                                if j >= 0:
                                    we = (j + 1) * P
                                    nc.vector.tensor_add(
                                        ss_full[:, off:we], ss_full[:, off:we],
                                        diag_sb[:, j, off:we])
                            else:
                                mt_t = mtp.tile([P, SC], F32, tag="mask")
                                nc.sync.dma_start(
                                    mt_t[:], maskT_d[t * P:(t + 1) * P, cq])
                                nc.vector.tensor_add(ss, ss, mt_t[:, off:])
                                nc.vector.tensor_scalar_max(ss, ss, -3.0e38)
                            e = epp.tile([P, SC], BF16, tag="e")
                            nc.scalar.activation(e[:, off:], ss, EXP)
                            return (t, off, e)

                        def flush(t, off, e):
                            nc.tensor.matmul(
                                pv[:, off:], v_sb[:, t, hs], e[:, off:],
                                start=(t == 0), stop=(t == nt - 1),
                                skip_group_check=True)
                            nc.tensor.matmul(
                                sm[:, off:], ones_sb[:], e[:, off:],
                                start=(t == 0), stop=(t == nt - 1),
                                skip_group_check=True)

                        pend = []
                        for t in range(nt):
                            pend.append(issue_scores(t))
                            if len(pend) > 2:
                                flush(*pend.pop(0))
                        for args in pend:
                            flush(*args)

                        rec = smp.tile([P, SC], F32, tag="rec")
                        nc.vector.reciprocal(rec[:], sm)
                        nc.vector.tensor_tensor(attn_c[:, h, :], pv, rec[:],
                                                MULT)

                    # o-proj for this sq chunk (all heads of attn_c ready)
                    for st in range(SC // P):
                        rs = slice(c * SC + st * P, c * SC + (st + 1) * P)
                        o_st = oop.tile([P, H], F32, tag="o_st", name="o_st")
                        for oc in range(H // SC):
                            ps = ps_mm.tile([P, SC], F32, tag="mm", name="ps_o")
                            for mt in range(NMT):
                                nc.tensor.matmul(
                                    ps, attn_c[:, mt, st * P:(st + 1) * P],
                                    wo_sb[:, mt, oc * SC:(oc + 1) * SC],
                                    start=(mt == 0), stop=(mt == NMT - 1))
                            nc.scalar.activation(
                                o_st[:, oc * SC:(oc + 1) * SC], ps, COPY)
                            if oc % 2 == 1:  # flush staged half early
                                hs_o = slice((oc - 1) * SC, (oc + 1) * SC)
                                nc.sync.dma_start(outp[rs, hs_o],
                                                  o_st[:, hs_o])

    nc.compile()
    return nc


_CACHE = {}


def _get_program(S, H, M, causal, repeat=1):
    key = (S, H, M, causal, repeat)
    if key not in _CACHE:
        _CACHE[key] = _build(S, H, M, causal, repeat=repeat)
    return _CACHE[key]


def _rope_tables(S, dim, base=10000.0):
    inv_freq = 1.0 / (base ** (np.arange(0, dim, 2, dtype=np.float64) / dim))
    t = np.arange(S, dtype=np.float64)
    freqs = np.outer(t, inv_freq)                     # [S, dim/2]
    emb = np.concatenate([freqs, freqs], axis=-1)     # [S, dim]
    return (np.cos(emb).astype(np.float32), np.sin(emb).astype(np.float32))


def _prep_in_maps(hidden_states, attention_mask, position_ids,
                  wq, wk, wv, wo):
    """Host-side shard + cast. Returns (in_maps, causal, M)."""
    Bq, S, H = hidden_states.shape
    assert Bq == B and H % HD == 0
    nh = H // HD
    groups = NCORES // B                     # head-groups per batch
    hpg = nh // groups                       # heads per core
    M = hpg * HD

    bf = ml_dtypes.bfloat16

    # causal-mask detection (exact match against the standard Llama pattern)
    neg = np.finfo(np.float32).min
    causal_ref = np.where(np.tril(np.ones((S, S), dtype=bool)),
                          np.float32(0.0), np.float32(neg))
    causal = all(np.array_equal(attention_mask[b, 0], causal_ref)
                 for b in range(B))

    cos_tab, sin_tab = _rope_tables(S, HD)
    scale = 1.0 / np.sqrt(HD)

    SC = 512
    P = 128
    in_maps = []
    for core in range(NCORES):
        b, g = divmod(core, groups)
        rows = slice(g * M, (g + 1) * M)
        x = hidden_states[b]                                   # [S, H]
        pos = position_ids[b].astype(np.int64)
        cosT = cos_tab[pos].T                                  # [HD, S]
        sinT = sin_tab[pos].T
        sinS = np.concatenate([-sinT[:HD // 2], sinT[HD // 2:]], axis=0)
        m = {
            "xT": np.ascontiguousarray(x.T).astype(bf),
            "wqT": np.ascontiguousarray((wq[rows].astype(np.float64) * scale)
                                        .astype(np.float32).T).astype(bf),
            "wkT": np.ascontiguousarray(wk[rows].T).astype(bf),
            "wvT": np.ascontiguousarray(wv[rows].T).astype(bf),
            "woT": np.ascontiguousarray(wo[:, rows].T).astype(bf),
            "trig": np.ascontiguousarray(
                np.stack([cosT, sinS], axis=1).astype(np.float32)),
        }
        if causal:
            p_idx = np.arange(P)[:, None, None]
            j_idx = np.arange(SC // P)[None, :, None]
            f_idx = np.arange(SC)[None, None, :]
            m["diag"] = np.where(p_idx + P * j_idx <= f_idx,
                                 np.float32(0.0),
                                 np.float32(NEG)).astype(np.float32)
        else:
            m["maskT"] = np.ascontiguousarray(attention_mask[b, 0].T)
        in_maps.append(m)
    return in_maps, causal, M


def kernel(hidden_states, attention_mask, position_ids, wq, wk, wv, wo):
    global LAST_RESULTS
    hidden_states = np.asarray(hidden_states, dtype=np.float32)
    attention_mask = np.asarray(attention_mask, dtype=np.float32)
    position_ids = np.asarray(position_ids)
    wq = np.asarray(wq, dtype=np.float32)
    wk = np.asarray(wk, dtype=np.float32)
    wv = np.asarray(wv, dtype=np.float32)
    wo = np.asarray(wo, dtype=np.float32)

    Bq, S, H = hidden_states.shape
    in_maps, causal, M = _prep_in_maps(
        hidden_states, attention_mask, position_ids, wq, wk, wv, wo)

    nc = _get_program(S, H, M, causal)
    globals()["LAST_IN_MAPS"] = in_maps
    res = run_bass_kernel_spmd(nc, in_maps, core_ids=list(range(NCORES)),
                               trace=False)
    LAST_RESULTS = res

    groups = NCORES // B
    out = np.zeros((B, S, H), dtype=np.float64)
    for core in range(NCORES):
        b = core // groups
        out[b] += res.results[core]["outp"].astype(np.float64)
    return out.astype(np.float32)
